# revision 6
# baseline (speedup 1.0000x reference)
"""Masked multi-head attention (B=4, S=2048, H=16, d_k=64) on 8 TRN2 NeuronCores.

Sharding: core c handles batch b = c//2 and head-group hg = c%2 (8 heads each,
processed as 4 pairs: head A on SBUF partitions 0-63, head B on 64-127).

v3 design (evolved from v2 via DIAG-mode differential measurements):
  * scores: bf16 row-tiled matmul pairs (64x128 PE tiles T0/T8 run the two
    heads CONCURRENTLY), [128, 1024] psum per head per kt (2 banks), single
    buffered - engines drain it during the interleaved attnV slot.
  * exp+mask in ONE engine pass per element, balanced 1/1/1 across engines:
      - head A (ACT+Pool): er = exp(psum/A) on scalar; e = er * mask on
        gpsimd.  mask resident as fp8e4m3 {0,1}.
      - head B (DVE): Schraudolph bits trick fused with the mask:
        e_bits_i16 = round(psum + B), B = 16384*m + 2048*(1-m) (fp8e5m2,
        both exactly representable), reinterpreted as bf16.  psum holds
        s*A (A = 128*log2 e; Q pre-scaled by A/8 on host).  B_UNMASK=16384
        = +128 bits vs the exp-exact 16256, an EXACT power-of-2 scale on
        all unmasked e - cancels between numerator and Z in the division.
        (numpy-emulated rel err 1.57e-2 vs harness ref, better than v2.)
  * attnV: [V | ones] stationary baked ON HOST (Z accumulates in psum rows
    64-127), chained over 16 k-tiles, emission delayed DELAY k-tiles so the
    PE interleaves next scores with previous attnV.
  * normalization on the HOST: kernel returns raw numerator rows 0-63 and
    Z row 64 per (head, q); numpy divides.
  * EVERYTHING is streamed per-window inside the measured loop (no serial
    resident-load phase): mask/bias as 2 span-window tiles each, V and K
    double-buffered per pair, Q per (pair, span).  A cold single call
    exposes almost no serial DMA.  Total HBM traffic ~19 MB/call/core.
"""

import sys

sys.path.insert(0, "/opt/trn_rl_repo")

import numpy as np
import ml_dtypes

import concourse.bass as bass
import concourse.tile as tile
import concourse.mybir as mybir
from concourse import bacc
from concourse import bass_utils

BF16 = mybir.dt.bfloat16
F16 = mybir.dt.float16
F32 = mybir.dt.float32
I16 = mybir.dt.int16
F8E5 = mybir.dt.float8e5
F8E4 = mybir.dt.float8e4

# Model dims
S = 2048
DK = 64
HPC = 8
N_CORES = 8
P = 128
W = 1024              # q-span width
KT_N = S // P         # 16
SPANS = S // W        # 2
PAIRS = HPC // 2      # 4
A_SCALE = 128.0 * np.log2(np.e)   # 184.6644

B_UNMASK = 16384.0    # fp8e5m2-exact; +128 bits = exact x2, cancels in num/Z
B_MASK = 2048.0       # keeps masked bits positive & tiny

DELAY = 3             # attnV emission delay in k-tiles
LAST_RESULTS = None
DIAG = None           # None | 'pe_only' | 'eng_only'


def build_program(reps=1):
    Exp = mybir.ActivationFunctionType.Exp
    hd = HPC * DK          # 512

    nc = bacc.Bacc("TRN2", debug=False)
    qT = nc.dram_tensor("qT", [hd, S], BF16, kind="ExternalInput").ap()
    kT = nc.dram_tensor("kT", [hd, S], BF16, kind="ExternalInput").ap()
    # V swizzled on host: [128, pair, 2 heads, 16 kt, 128(=[V|ones])] bf16
    vw = nc.dram_tensor("vw", [P, PAIRS * 2 * KT_N * P], BF16,
                        kind="ExternalInput").ap()
    # mask/bias, span-major: [128, span, kt, 1024]
    m8 = nc.dram_tensor("m8", [P, SPANS * KT_N * W], F8E4,
                        kind="ExternalInput").ap()
    b8 = nc.dram_tensor("b8", [P, SPANS * KT_N * W], F8E5,
                        kind="ExternalInput").ap()
    out_raw = nc.dram_tensor("out_raw", [HPC, 65, S], F32,
                             kind="ExternalOutput").ap()

    with tile.TileContext(nc) as tc:
        with (
            tc.tile_pool(name="mwin", bufs=1) as mwin,
            tc.tile_pool(name="bwin", bufs=1) as bwin,
            tc.tile_pool(name="vwin", bufs=2) as vwin,
            tc.tile_pool(name="kwinp", bufs=2) as kwinp,
            tc.tile_pool(name="qwinp", bufs=2) as qwinp,
            tc.tile_pool(name="erp", bufs=2) as erp,
            tc.tile_pool(name="ep", bufs=10) as ep,
            tc.tile_pool(name="osbp", bufs=4) as osbp,
            tc.tile_pool(name="psA", bufs=1, space="PSUM") as psA,
            tc.tile_pool(name="psB", bufs=1, space="PSUM") as psB,
            tc.tile_pool(name="poA", bufs=1, space="PSUM") as poA,
            tc.tile_pool(name="poB", bufs=1, space="PSUM") as poB,
        ):
            # span-window tiles for mask/bias (allocated once, DMA'd per rep)
            m_sb = [mwin.tile([P, KT_N * W], F8E4, tag=f"m{sp}",
                              name=f"m_sb{sp}")
                    for sp in range(SPANS)]
            b_sb = [bwin.tile([P, KT_N * W], F8E5, tag=f"b{sp}",
                              name=f"b_sb{sp}")
                    for sp in range(SPANS)]

            for rep in range(reps):
                for sp in range(SPANS):
                    nc.sync.dma_start(
                        m_sb[sp][:], m8[:, sp * KT_N * W:(sp + 1) * KT_N * W])
                    nc.sync.dma_start(
                        b_sb[sp][:], b8[:, sp * KT_N * W:(sp + 1) * KT_N * W])

                for p in range(PAIRS):
                    # [V|ones] stationaries for this pair's 2 heads, 16 kts
                    vbuf = vwin.tile([P, 2 * KT_N * P], BF16, tag="vw")
                    nc.sync.dma_start(
                        vbuf[:], vw[:, p * 2 * KT_N * P:(p + 1) * 2 * KT_N * P])
                    v3 = vbuf.rearrange("p (s e) -> p s e", e=P)
                    kwin = kwinp.tile([P, S], BF16, tag="kw")
                    nc.sync.dma_start(kwin[:], kT[p * P:(p + 1) * P, :])

                    for sp in range(SPANS):
                        qwin = qwinp.tile([P, W], BF16, tag="qw")
                        nc.sync.dma_start(
                            qwin[:],
                            qT[p * P:(p + 1) * P, sp * W:(sp + 1) * W])
                        if DIAG != "eng_only":
                            o_psA = poA.tile([P, W], F32, tag="oA")
                            o_psB = poB.tile([P, W], F32, tag="oB")
                        hA, hB = 2 * p, 2 * p + 1

                        pending = []   # delayed attnV thunks, kt granularity

                        def emit_attnv(kt, eA, eB):
                            def go():
                                for hf in range(2):
                                    cs = slice(hf * 512, (hf + 1) * 512)
                                    nc.tensor.matmul(
                                        o_psA[:, cs],
                                        lhsT=v3[:, kt, :],
                                        rhs=eA[:, cs],
                                        start=(kt == 0), stop=(kt == KT_N - 1))
                                    nc.tensor.matmul(
                                        o_psB[:, cs],
                                        lhsT=v3[:, KT_N + kt, :],
                                        rhs=eB[:, cs],
                                        start=(kt == 0), stop=(kt == KT_N - 1))
                            return go

                        for kt in range(KT_N):
                            if DIAG != "eng_only" and len(pending) > DELAY:
                                pending.pop(0)()
                            s_psA = psA.tile([P, W], F32, tag="sA")
                            s_psB = psB.tile([P, W], F32, tag="sB")
                            for hf in range(2):
                                cs = slice(hf * 512, (hf + 1) * 512)
                                nc.tensor.matmul(
                                    s_psA[:, cs],
                                    lhsT=kwin[0:64, kt * P:(kt + 1) * P],
                                    rhs=qwin[0:64, cs], start=True, stop=True)
                                nc.tensor.matmul(
                                    s_psB[:, cs],
                                    lhsT=kwin[64:128, kt * P:(kt + 1) * P],
                                    rhs=qwin[64:128, cs], start=True, stop=True)
                            eA = ep.tile([P, W], BF16, tag="e")
                            eB = ep.tile([P, W], BF16, tag="e")
                            if DIAG == "pe_only":
                                if rep == 0 and p == 0 and sp == 0 and kt < 5:
                                    nc.gpsimd.memset(eA[:], 0.001)
                                    nc.gpsimd.memset(eB[:], 0.001)
                                nc.vector.tensor_copy(eA[0:1, 0:8],
                                                      s_psA[0:1, 0:8])
                                nc.vector.tensor_copy(eB[0:1, 0:8],
                                                      s_psB[0:1, 0:8])
                            else:
                                # head A: exact exp (ACT) then fp8-mask mul
                                # (Pool); head B: DVE bits trick w/ fp8 bias
                                erA = erp.tile([P, W], BF16, tag="er")
                                nc.scalar.activation(
                                    erA[:], s_psA[:], Exp,
                                    scale=float(1.0 / A_SCALE))
                                nc.gpsimd.tensor_mul(
                                    eA[:], erA[:],
                                    m_sb[sp][:, kt * W:(kt + 1) * W])
                                nc.vector.tensor_add(
                                    eB[:].bitcast(I16), s_psB[:],
                                    b_sb[sp][:, kt * W:(kt + 1) * W])
                            if DIAG != "eng_only":
                                pending.append(emit_attnv(kt, eA, eB))
                        for go in pending:
                            go()
                        # evac: numerator rows 0-63 + Z row 64
                        o_sbA = osbp.tile([65, W], F32, tag="osb")
                        o_sbB = osbp.tile([65, W], F32, tag="osb")
                        if DIAG == "eng_only":
                            nc.vector.memset(o_sbA[:], 0.0)
                            nc.vector.memset(o_sbB[:], 0.0)
                        else:
                            nc.scalar.copy(o_sbA[:], o_psA[0:65, :])
                            nc.vector.tensor_copy(o_sbB[:], o_psB[0:65, :])
                        nc.sync.dma_start(
                            out_raw[hA, :, sp * W:(sp + 1) * W], o_sbA[:])
                        nc.sync.dma_start(
                            out_raw[hB, :, sp * W:(sp + 1) * W], o_sbB[:])
    nc.compile()
    return nc


_PROG = None


def _get_prog():
    global _PROG
    if _PROG is None:
        _PROG = build_program()
    return _PROG


def _prep_in_maps(query, key, value, mask):
    query = np.asarray(query, dtype=np.float32)
    key = np.asarray(key, dtype=np.float32)
    value = np.asarray(value, dtype=np.float32)
    mask = np.asarray(mask)
    B = query.shape[0]
    bf16 = ml_dtypes.bfloat16
    f8e4 = ml_dtypes.float8_e4m3
    f8e5 = ml_dtypes.float8_e5m2
    hd = HPC * DK

    m8s, b8s = [], []
    for b in range(B):
        mt = np.ascontiguousarray(mask[b, 0].T)         # [k, q] int
        # span-major [128, span, kt, 1024]
        m4 = mt.reshape(KT_N, P, SPANS, W).transpose(1, 2, 0, 3)
        m8s.append(np.ascontiguousarray(
            m4.astype(np.float32)).astype(f8e4).reshape(P, -1))
        b4 = np.where(m4 != 0, np.float32(B_UNMASK), np.float32(B_MASK))
        b8s.append(b4.astype(f8e5).reshape(P, -1))

    q_scale = A_SCALE / 8.0
    in_maps = []
    for c in range(N_CORES):
        b, hg = divmod(c, 2)
        cols = slice(hg * hd, (hg + 1) * hd)
        vb = value[b][:, cols]                          # [S, 512]
        # [V|ones] slots: [128 part, pair, head, kt, 128]
        vsw = np.ones((P, PAIRS, 2, KT_N, P), np.float32)
        vr = vb.reshape(KT_N, P, HPC, DK).transpose(1, 2, 0, 3)  # [p,h,kt,dk]
        vsw[:, :, :, :, 0:DK] = vr.reshape(P, PAIRS, 2, KT_N, DK)
        in_maps.append({
            "qT": np.ascontiguousarray(
                (query[b][:, cols] * q_scale).T).astype(bf16),
            "kT": np.ascontiguousarray(key[b][:, cols].T).astype(bf16),
            "vw": vsw.astype(bf16).reshape(P, -1),
            "m8": m8s[b],
            "b8": b8s[b],
        })
    return in_maps


def _unshard(results, B, s, D):
    hd = HPC * DK
    out = np.empty((B, s, D), np.float32)
    for c in range(N_CORES):
        b, hg = divmod(c, 2)
        raw = results[c]["out_raw"]          # [8, 65, s]
        num = raw[:, 0:64, :]                # [8, 64, s]
        z = raw[:, 64:65, :]                 # [8, 1, s]
        o = (num / z).transpose(2, 0, 1).reshape(s, hd)   # [s, hd]
        out[b][:, hg * hd:(hg + 1) * hd] = o
    return out


def kernel(query, key, value, mask):
    global LAST_RESULTS
    B, s, D = np.asarray(query).shape
    in_maps = _prep_in_maps(query, key, value, mask)
    nc = _get_prog()
    res = bass_utils.run_bass_kernel_spmd(
        nc, in_maps, core_ids=list(range(N_CORES)), trace=False)
    LAST_RESULTS = res
    return _unshard(res.results, B, s, D)
